# revision 2
# baseline (speedup 1.0000x reference)
"""Causal GQA attention (paged-KV prefill) distributed over 8 TRN2 NeuronCores.

Problem: q [4096,16,128], k/v [4096,4,128] packed as B=2 sequences of S=2048,
KV-cache scatter via slot_mapping then causal attention (GQA group 4).

Sharding: 8 cores = (B=2) x (Hkv=4). Core c handles batch c//4, kv-head c%4
with its 4 query heads. No cross-core communication needed.

Per-core kernel (Bass/Tile):
  - host pre-casts shards to bf16 and pre-TRANSPOSES K and Q to the
    [d=128, seq] layout the PE needs (head_dim on partitions), packing all
    k-tiles and q-quarters into ONE DRAM array in first-use order; SBUF
    loads are then plain full-line-rate DMA prefixes staged so the first
    score matmul's operands land ~2.6us in and later tiles always arrive
    ahead of use (no xbar transposes at all)
  - v arrives padded to 129 cols with a ones column baked in
  - scores^T tile [k=128, q<=512] = kT_tile.T @ qT_chunk on TensorE (bf16),
    causally trimmed: diagonal-band tiles only compute the valid query range
  - exp(scale*s) on ScalarE straight out of PSUM, one call per 2-tile
    group; every 2nd fully-causal group instead computes exp on VectorE as
    an int16 affine whose bits are bf16(exp(x)) (Schraudolph), offloading
    the otherwise-saturated ScalarE (~5e-3 extra end-to-end error)
  - causal diag blocks masked via 0/1 triangular mult on VectorE
  - out accumulation: psum_o[q=128, 129] += probT_tile.T @ [v_tile | 1],
    the 129th column accumulates the softmax denominator for free; two
    q-subblocks pack into one PSUM bank ([128, 258]). Each bank's first
    AV opens the 2KB zero region with start=True; the bank's second
    accumulator then overwrites its has_written=0 region (two interleaved
    start-groups in one bank would clear each other's has_written bits)
  - normalize: directly from PSUM (VectorE reciprocal + tensor_scalar),
    per PSUM bank as soon as that bank's last AV retires -- bank A (q-sub
    0,1) one group before the chunk ends -- then DMA the half-chunk out;
    this frees po banks early, halves the chunk-boundary stall, and avoids
    the PSUM->SBUF staging copies entirely
  - emission is software-pipelined with a 4-unit scores lookahead (PE
    always has queued score matmuls while ScalarE/VectorE exp a group),
    q-chunks run largest-first, and the lookahead drains before the final
    chunk to shorten the end-of-kernel tail

PSUM budget (8 banks): scores [128,1024] x3 bufs = 6, packed out
accumulators [128,258] x2 tags x1 buf = 2.
"""

import os
import sys

import numpy as np

for _p in ("/opt/trn_rl_repo",):
    if os.path.isdir(_p) and _p not in sys.path:
        sys.path.insert(0, _p)

import ml_dtypes  # noqa: E402

from concourse import bass, bacc, mybir, tile  # noqa: E402
from concourse.bass_utils import run_bass_kernel_spmd  # noqa: E402

B, S, H, HKV, D = 2, 2048, 16, 4, 128
GRP = H // HKV  # query heads per kv head
NCORES = 8
ST = S // 128  # 16 k-tiles of 128
QB = S // 512  # 4 q-chunks of 512
SCALE = 0.08838834764831845  # 1/sqrt(128)
# Schraudolph-in-bf16-bits exp on DVE: int16 bits = A16*(scale*s) + B16
# approximate bf16(exp(scale*s)) to ~3% per element. Applied to every
# DVE_EVERY-th fully-below-diagonal score group to offload the saturated
# ScalarE; softmax renormalization cancels most of the per-element error.
import math as _math

A16S = (2.0**7) / _math.log(2.0) * SCALE
B16 = 127.0 * 2**7 - 366393.0 / 2**16
DVE_EVERY = 2

F32 = mybir.dt.float32
BF16 = mybir.dt.bfloat16
I16 = mybir.dt.int16

_CACHED_NC = None

# ---------------------------------------------------------------------------
# kq packed-column layout: all of kT (16 tiles) and the four heads' qT
# (16 tiles each), in first-use order so staged DMA prefixes always arrive
# ahead of the emission schedule. Tile unit = 128 columns.
#   [K0 K1 | Q0_qb3 | K2..K5 | K6..K11 | K12..K15 | Q0_qb2 Q0_qb1 Q0_qb0 |
#    Q1 (qb3,2,1,0) | Q2 ... | Q3 ...]
_K_SLOT = [0, 1, 6, 7, 8, 9, 10, 11, 12, 13, 14, 15, 16, 17, 18, 19]


def _kcol(t):
    return _K_SLOT[t] * 128


def _qbase(h, qb):
    """Column (in units of 128) of q-tile 4*qb for head h."""
    if h == 0:
        return {3: 2, 2: 20, 1: 24, 0: 28}[qb] * 128
    base = 32 + 16 * (h - 1)
    return (base + 4 * (3 - qb)) * 128


KQ_COLS = 80 * 128  # 16 k-tiles + 4 heads x 16 q-tiles

# staged input DMA plan: kq column ranges (tile units), interleaved with v
# (tile ranges of the [128, ST, 129] layout) and the tri mask
_DMA_PLAN = [
    ("kq", 0, 6),  # K0,K1 + Q0 qb3: first score group's operands
    ("kq", 6, 10),  # K2..K5
    ("v", 0, 4),  # V tiles for the first AVs
    ("kq", 10, 16),  # K6..K11
    ("tri", 0, 0),
    ("kq", 16, 20),  # K12..K15
    ("v", 4, 10),
    ("kq", 20, 32),  # Q0 qb2,qb1,qb0
    ("v", 10, 16),
    ("kq", 32, 48),  # Q1
    ("kq", 48, 64),  # Q2
    ("kq", 64, 80),  # Q3
]


def _build_graph():
    nc = bacc.Bacc(
        "TRN2", target_bir_lowering=False, debug=False, num_devices=NCORES
    )
    kq_ext = nc.declare_dram_parameter("kq", [128, KQ_COLS], BF16, isOutput=False)
    v_ext = nc.declare_dram_parameter("v", [128, ST, D + 1], BF16, isOutput=False)
    tri_ext = nc.declare_dram_parameter("tri", [128, 128], BF16, isOutput=False)
    out_ext = nc.declare_dram_parameter("out", [S, GRP, D], F32, isOutput=True)

    with tile.TileContext(nc) as tc:
        with (
            tc.tile_pool(name="const", bufs=1) as constp,
            tc.tile_pool(name="kv", bufs=1) as kvp,
            tc.tile_pool(name="prob", bufs=10) as probp,
            tc.tile_pool(name="osb", bufs=8) as osbp,
            tc.tile_pool(name="small", bufs=16) as smallp,
            tc.tile_pool(name="ps_s", bufs=3, space=bass.MemorySpace.PSUM) as pss,
            tc.tile_pool(name="ps_o", bufs=1, space=bass.MemorySpace.PSUM) as pso,
        ):
            # 0/1 lower-allowed mask for diagonal blocks: tri[kk, qq] = kk <= qq
            tri = constp.tile([128, 128], BF16)
            kq = kvp.tile([128, KQ_COLS], BF16, tag="kq")
            kqf = kq[:]
            v_aug = kvp.tile([128, ST, 129], BF16, tag="vaug")
            v_augf = v_aug[:].rearrange("s0 st d -> s0 (st d)")

            # staged input loads, first-use order (plain copies, no xbar)
            for kind, a, b_ in _DMA_PLAN:
                if kind == "kq":
                    nc.sync.dma_start(
                        kq[:, a * 128 : b_ * 128], kq_ext.ap()[:, a * 128 : b_ * 128]
                    )
                elif kind == "v":
                    nc.sync.dma_start(v_aug[:, a:b_, :], v_ext.ap()[:, a:b_, :])
                else:
                    nc.sync.dma_start(tri[:], tri_ext.ap())

            # warm the exp table set while input DMAs run
            warm = smallp.tile([128, 1], F32, tag="warm")
            nc.vector.memset(warm[:], 0.0)
            nc.scalar.activation(
                warm[:], warm[:], mybir.ActivationFunctionType.Exp
            )
            # warm the PE clock (HAM ramps over ~3.4us of sustained
            # activity): stream dummy matmuls while the first input DMAs
            # are still in flight so the real scores start at full rate
            wmm = smallp.tile([128, 8], BF16, tag="wmm")
            nc.gpsimd.memset(wmm[:], 0.0)
            wps = pss.tile([128, 1024], F32, tag="s", name="wps")
            for _ in range(150):
                nc.tensor.matmul(
                    wps[:8, 0:8],
                    wmm[:],
                    wmm[:],
                    start=True,
                    stop=True,
                )

            # out view: q index decomposes as qb*512 + bk*256 + jj*128 + s0
            outr = out_ext.ap().rearrange(
                "(qb bk jj s0) h d -> qb h bk s0 jj d", bk=2, jj=2, s0=128
            )

            def po_slice(po, j):
                t = po[0] if j < 2 else po[1]
                off = 129 * (j % 2)
                return t[:, off : off + 129]

            def emit_scores(h, qb, g):
                """Issue the two trimmed score matmuls for k-tile pair g."""
                kbs = (2 * g, 2 * g + 1)
                trims = [max(0, kb - 4 * qb) * 128 for kb in kbs]
                widths = [512 - t for t in trims]
                same_bank = widths[0] + widths[1] <= 512
                # same-bank trimmed pair packs contiguously: tile0's
                # start=True pending-zeroes the whole bank, tile1 writes
                # its slice with start=False (overwrite of pending bytes),
                # so the exp reads one contiguous hole-free range
                offs = [0, widths[0]] if same_bank else [0, 512]
                ps = pss.tile([128, 1024], F32, tag="s", name="ps")
                qstart = _qbase(h, qb)
                for i in (0, 1):
                    kb, t, w, o = kbs[i], trims[i], widths[i], offs[i]
                    nc.tensor.matmul(
                        ps[:, o : o + w],
                        kqf[:, _kcol(kb) : _kcol(kb) + 128],
                        kqf[:, qstart + t : qstart + 512],
                        start=(not same_bank) or i == 0,
                        stop=(not same_bank) or i == 1,
                    )
                return (ps, kbs, trims, offs, widths)

            def norm_and_store(po, bk, h, qb):
                """Normalize one PSUM out bank (2 q-subblocks) straight from
                PSUM and DMA the half-chunk out."""
                out_sb = osbp.tile([128, 2, 128], F32, tag="out", name="osb")
                for jj in (0, 1):
                    aj = po[bk][:, 129 * jj : 129 * jj + 129]
                    rcp = smallp.tile([128, 1], F32, tag="rcp", name="rcp")
                    nc.vector.reciprocal(rcp[:], aj[:, 128:129])
                    nc.vector.tensor_scalar_mul(
                        out_sb[:, jj, :], aj[:, 0:128], rcp[:]
                    )
                nc.sync.dma_start(outr[qb, h, bk], out_sb[:])

            def emit_rest(h, qb, g, po, scored):
                """exp + mask + AV accumulation for a scored group; normalize
                + store each out bank as soon as its accumulation closes."""
                ps, kbs, trims, offs, widths = scored
                # DVE-exp: every DVE_EVERY-th fully-causal group
                full = kbs[1] < 4 * qb  # both tiles fully below the diagonal
                if full:
                    exp_state["ctr"] += 1
                if full and exp_state["ctr"] % DVE_EVERY == 1:
                    # offload this group's exp to DVE (Schraudolph bf16 bits)
                    i16 = probp.tile([128, 1024], I16, tag="p", name="probTi")
                    nc.vector.tensor_scalar(
                        i16[:],
                        ps[:],
                        A16S,
                        B16,
                        mybir.AluOpType.mult,
                        mybir.AluOpType.add,
                    )
                    probT = i16.bitcast(BF16)
                else:
                    probT_t = probp.tile(
                        [128, 1024], BF16, tag="p", name="probT"
                    )
                    probT = probT_t[:]
                    total_w = offs[1] + widths[1]  # contiguous, hole-free
                    nc.scalar.activation(
                        probT[:, 0:total_w],
                        ps[:, 0:total_w],
                        mybir.ActivationFunctionType.Exp,
                        scale=SCALE,
                    )
                started_banks = set()
                for i in (0, 1):
                    kb, t, o = kbs[i], trims[i], offs[i]
                    j0 = t // 128
                    diag = kb >= 4 * qb
                    if diag:  # diagonal tile: mask its first q-block
                        blk = probT[:, o : o + 128]
                        nc.vector.tensor_mul(blk, blk, tri[:])
                    # masked block's AV last so it doesn't wait on the DVE
                    js = list(range(j0 + 1, 4)) + [j0] if diag else range(4)
                    for j in js:
                        qsub = 4 * qb + j
                        co = o + (j - j0) * 128
                        # The first AV (in emission order) touching each
                        # bank at kb=0 opens its zero region with start=True
                        # (clears has_written for the whole 2KB bank); the
                        # bank's other accumulator then lands on
                        # has_written=0 and overwrites. Only the bank's last
                        # AV carries stop.
                        bank = j // 2
                        start = kb == 0 and bank not in started_banks
                        if kb == 0:
                            started_banks.add(bank)
                        nc.tensor.matmul(
                            po_slice(po, j),
                            probT[:, co : co + 128],
                            v_augf[:, kb * 129 : (kb + 1) * 129],
                            start=start,
                            stop=(j % 2 == 1 and kb == qsub),
                            skip_group_check=True,
                        )
                if g == 2 * qb:  # bank A (q-sub 0,1) closed: store it now
                    norm_and_store(po, 0, h, qb)
                if g == 2 * qb + 1:  # bank B closed: last group of the chunk
                    norm_and_store(po, 1, h, qb)

            # Software-pipelined emission: issue scores(u+1) before the
            # exp-dependent work of unit u so PE never waits on ACT.
            exp_state = {"ctr": 0}
            pending = []  # scores lookahead (ps_s has 3 bufs)
            order = [
                (h, qb) for h in range(GRP) for qb in (3, 2, 1, 0)
            ]  # big chunks first within each head, small-drain tail
            for h, qb in order:
                    if qb == 0:
                        # drain the lookahead before each small chunk: its
                        # diag-heavy groups contend for ps slots with the
                        # queued units (flush fully before the last chunk)
                        keep = 1 if h == GRP - 1 else 2
                        while len(pending) > keep:
                            emit_rest(*pending.pop(0))
                    # packed out accumulators: bank A holds q-subblocks 0,1
                    # at cols [0,129)/[129,258); bank B holds 2,3.
                    po01 = pso.tile([128, 258], F32, tag="o01", name="po01")
                    po23 = pso.tile([128, 258], F32, tag="o23", name="po23")
                    po = (po01, po23)
                    for g in range(2 * qb + 2):
                        scored = emit_scores(h, qb, g)
                        pending.append((h, qb, g, po, scored))
                        if len(pending) > 4:
                            emit_rest(*pending.pop(0))
            for p in pending:
                emit_rest(*p)

    nc.compile()
    return nc


def _get_nc():
    global _CACHED_NC
    if _CACHED_NC is None:
        _CACHED_NC = _build_graph()
    return _CACHED_NC


def _effective_kv(kv, cache, slot):
    """Mirror reference _store_kvcache + gather: returns cache-after-scatter
    gathered at slot positions, shape [B, S, HKV, D]."""
    valid = slot >= 0
    safe = np.where(valid, slot, 0)
    cache = np.array(cache, dtype=np.float32, copy=True)
    val = np.where(valid[:, None, None], kv, cache[safe])
    cache[safe] = val
    return cache[safe.reshape(B, S)]


def _tile_sd(x):
    """[S, D] -> [128, ST, D] with row s at [s % 128, s // 128]."""
    S_, D_ = x.shape
    return np.ascontiguousarray(
        x.reshape(S_ // 128, 128, D_).transpose(1, 0, 2)
    )


def _prep_core_inputs(qb, kk, vv, tri, c):
    bf16 = ml_dtypes.bfloat16
    b, g = c // HKV, c % HKV
    q_sh = qb[b, :, g * GRP : (g + 1) * GRP, :].astype(bf16)  # [S, GRP, D]
    k_sh = kk[b, :, g, :].astype(bf16)  # [S, D]
    kq = np.empty((128, KQ_COLS), dtype=bf16)
    kT = np.ascontiguousarray(k_sh.T)  # [128 d, S]
    for t in range(ST):
        kq[:, _kcol(t) : _kcol(t) + 128] = kT[:, t * 128 : (t + 1) * 128]
    for h in range(GRP):
        qT = np.ascontiguousarray(q_sh[:, h, :].T)  # [128 d, S]
        for qbi in range(QB):
            c0 = _qbase(h, qbi)
            kq[:, c0 : c0 + 512] = qT[:, qbi * 512 : (qbi + 1) * 512]
    v_sd = vv[b, :, g, :].astype(bf16)  # [S, D]
    v_pad = np.concatenate(
        [v_sd, np.ones((S, 1), dtype=bf16)], axis=1
    )  # ones col baked in
    v_tiled = _tile_sd(v_pad)
    return {"kq": kq, "v": v_tiled, "tri": tri}


def kernel(q, k, v, k_cache, v_cache, slot_mapping, batch, seqlen, **_ignored):
    q = np.asarray(q, dtype=np.float32)
    k = np.asarray(k, dtype=np.float32)
    v = np.asarray(v, dtype=np.float32)
    slot = np.asarray(slot_mapping).astype(np.int64)
    assert int(batch) == B and int(seqlen) == S
    assert q.shape == (B * S, H, D)

    kk = _effective_kv(k, k_cache, slot)  # [B, S, HKV, D]
    vv = _effective_kv(v, v_cache, slot)
    qb = q.reshape(B, S, H, D)

    tri = np.triu(np.ones((128, 128), dtype=np.float32)).astype(
        ml_dtypes.bfloat16
    )

    in_maps = [
        _prep_core_inputs(qb, kk, vv, tri, c) for c in range(NCORES)
    ]

    nc = _get_nc()
    res = run_bass_kernel_spmd(nc, in_maps, core_ids=list(range(NCORES)))

    out = np.empty((B, S, H, D), dtype=np.float32)
    for c in range(NCORES):
        b, g = c // HKV, c % HKV
        out[b, :, g * GRP : (g + 1) * GRP, :] = res.results[c]["out"]
    return out.reshape(B * S, H, D)
